# revision 21
# baseline (speedup 1.0000x reference)
"""Trainium2 Bass kernel for nn_CaptionHead (segment_reduce).

Math (reference):
    feats = adapter_feats[v2p_map]                      # [N_pts, D]
    fhat  = feats / max(||feats||, eps)
    scores = (fhat @ E.T) * exp(logit_scale)            # [N_pts, C]
    logp = log_softmax(scores, -1)
    pooled[c] = mean_{t in seg c} logp[p_t]             # [C, C]
    counts[c] = |seg c|

Key algebraic restructuring: with s = scale * fhat,
    logp[p, :] = (s_p @ E.T) - lse_p,   lse_p = log(sum_c exp(s_p . e_c))
so the segment mean decomposes into
    pooled[c] = ( (sum_t s_{p_t}) @ E.T - sum_t lse_{p_t} ) / count_c
which needs only per-point rows + one [C_loc, D] accumulator per core —
the [N_pts, C] log-prob matrix is never materialized.  The segment sum
itself is a matmul against a (host-built) caption-x-row multiplicity
matrix W, so the device never does an indexed gather at all.

Two SPMD passes over 8 cores (host does the indexing/sharding between):
  k1 (point-parallel): each core streams its N_pts/8 pre-gathered voxel
     rows, normalizes, computes lse via an on-chip matmul vs the caption
     embeddings, and writes [fhat*scale | lse | 1] rows (514 bf16) back.
     Point-parallel means every point's lse is computed exactly once
     (caption-parallel would recompute shared points, T/N_pts = 1.75x).
  k2 (caption-parallel): each core owns 128 captions and computes
     [G | X] = W @ [rows], pooled = (G @ E.T - lse_sum) / count, where
     W[c, r] = multiplicity of compacted row r in caption c's segment.
"""

import math
from contextlib import ExitStack

import ml_dtypes
import numpy as np

import concourse.bacc as bacc
import concourse.bass as bass
import concourse.tile as tile
from concourse import mybir
from concourse.bass_utils import run_bass_kernel_spmd
from concourse.masks import make_identity

N_CORES = 8
P = 128
D = 512
C_TOT = 1024
C_LOC = C_TOT // N_CORES
DK = D // P            # 128-row d-chunks
ROW = D + 2            # frow width: fhat | lse | 1

BF16 = mybir.dt.bfloat16
F32 = mybir.dt.float32
FP8 = mybir.dt.float8e4
NP_FP8 = mybir.dt.np(FP8)
AF = mybir.ActivationFunctionType
ALU = mybir.AluOpType

_BUILD_CACHE: dict = {}

# Set by a test harness (with an NTFF profile hook installed) to collect
# per-pass HW exec times; leave False in normal use.
TRACE = False
LAST_EXEC_NS = None


def _prefer_ln_exp_table():
    """Make the act-table-set selector favor `natural_log_exp_and_others`,
    which contains both Ln and Exp — otherwise Bacc alternates between the
    Ln-only and Exp-only sets, inserting an ACT table load (~2.7us) around
    nearly every activation."""
    import concourse.bacc as _bacc_mod

    orig = _bacc_mod.get_activation_tables
    if getattr(orig, "_lnexp_patched", False):
        return

    def patched(arch):
        t = dict(orig(arch))
        pref = "natural_log_exp_and_others"
        if pref in t:
            # Keep set order/indices intact (walrus emits the index), but
            # make `pref` the only set advertising Ln/Exp.
            drop = {AF.Ln, AF.Exp}
            t = {k: (v if k == pref else (set(v) - drop)) for k, v in t.items()}
        return t

    patched._lnexp_patched = True
    _bacc_mod.get_activation_tables = patched


K1_GRP = 4  # point-tiles per DMA/activation group


def _build_k1(t1: int):
    """Point pass: stream pre-gathered rows -> normalize -> lse -> frows.

    Tiles are processed in groups of K1_GRP: one input DMA, one output DMA,
    and batched [P, K1_GRP] activations (Ln / Exp) per group, so neither
    the SP DMA-issue cost nor the ACT per-op overhead is paid per tile.
    """
    _prefer_ln_exp_table()
    nc = bacc.Bacc("TRN2", target_bir_lowering=False, debug=False,
                   num_devices=N_CORES)
    assert t1 % K1_GRP == 0
    n_pt = t1 * P
    n_grp = t1 // K1_GRP
    gp = K1_GRP * P

    pf_in = nc.dram_tensor("pf", [n_pt, D], BF16, kind="ExternalInput")
    et_in = nc.dram_tensor("et", [P, DK, C_TOT], BF16, kind="ExternalInput")
    ls_in = nc.dram_tensor("ls", [1, 1], F32, kind="ExternalInput")
    frows_out = nc.dram_tensor("frows", [n_pt, ROW], BF16, kind="ExternalOutput")

    with tile.TileContext(nc) as tc, ExitStack() as ctx:
        outer = ctx.enter_context(tc.tile_pool(name="outer", bufs=1))
        et_sb = outer.tile([P, DK, C_TOT], BF16)
        nc.sync.dma_start(out=et_sb[:], in_=et_in[:])
        ident = outer.tile([P, P], BF16)
        make_identity(nc, ident[:])
        ls_sb = outer.tile([P, 1], F32)
        nc.gpsimd.dma_start(out=ls_sb[:], in_=ls_in[:].to_broadcast([P, 1]))

        gpool = ctx.enter_context(tc.tile_pool(name="gath", bufs=3))
        spool = ctx.enter_context(tc.tile_pool(name="work", bufs=3))
        stat = ctx.enter_context(tc.tile_pool(name="stat", bufs=3))
        opool = ctx.enter_context(tc.tile_pool(name="orow", bufs=3))
        psum_s = ctx.enter_context(tc.tile_pool(name="ps_s", bufs=2, space="PSUM"))
        psum_t = ctx.enter_context(tc.tile_pool(name="ps_t", bufs=2, space="PSUM"))

        for g in range(n_grp):
            f_grp = gpool.tile([P, K1_GRP, D], BF16, tag="f")
            nc.sync.dma_start(
                out=f_grp[:],
                in_=pf_in[g * gp : (g + 1) * gp, :].rearrange(
                    "(b p) d -> p b d", p=P
                ),
            )
            frow_grp = opool.tile([P, K1_GRP, ROW], BF16, tag="frow")

            # batched sum-of-squares -> rns = exp(ls - 0.5*ln(ss))
            ss4 = stat.tile([P, K1_GRP], F32, tag="ss")
            for jj in range(K1_GRP):
                fsq = spool.tile([P, D], BF16, tag="fsq")
                nc.gpsimd.tensor_mul(fsq[:], f_grp[:, jj, :], f_grp[:, jj, :])
                nc.vector.reduce_sum(
                    out=ss4[:, jj : jj + 1], in_=fsq[:], axis=mybir.AxisListType.X
                )
            lnss4 = stat.tile([P, K1_GRP], F32, tag="lnss")
            nc.scalar.activation(out=lnss4[:], in_=ss4[:], func=AF.Ln)
            rns4 = stat.tile([P, K1_GRP], F32, tag="rns")
            nc.scalar.activation(
                out=rns4[:], in_=lnss4[:], func=AF.Exp, scale=-0.5, bias=ls_sb[:, 0:1]
            )

            se4 = stat.tile([P, K1_GRP], F32, tag="se")
            for jj in range(K1_GRP):
                frow = frow_grp[:, jj, :]
                nc.vector.tensor_scalar_mul(
                    frow[:, 0:D], f_grp[:, jj, :], rns4[:, jj : jj + 1]
                )

                t_ps = psum_t.tile([P, D], BF16, tag="tps")
                for k in range(DK):
                    nc.tensor.transpose(
                        out=t_ps[:, k * P : (k + 1) * P],
                        in_=frow[:, k * P : (k + 1) * P],
                        identity=ident[:],
                    )
                ft = spool.tile([P, D], BF16, tag="ft")
                nc.vector.tensor_copy(out=ft[:], in_=t_ps[:])

                s_ps = psum_s.tile([P, C_TOT], F32, tag="sps")
                for k in range(DK):
                    for h in range(2):
                        nc.tensor.matmul(
                            out=s_ps[:, h * 512 : (h + 1) * 512],
                            lhsT=ft[:, k * P : (k + 1) * P],
                            rhs=et_sb[:, k, h * 512 : (h + 1) * 512],
                            start=(k == 0),
                            stop=(k == DK - 1),
                        )
                nc.scalar.activation(
                    out=s_ps[:], in_=s_ps[:], func=AF.Exp,
                    accum_out=se4[:, jj : jj + 1],
                )

            # batched lse + ones columns (strided APs into the group tile)
            nc.scalar.activation(
                out=frow_grp[:, :, D : D + 1].rearrange("p b one -> p (b one)"),
                in_=se4[:],
                func=AF.Ln,
            )
            nc.vector.memset(frow_grp[:, :, D + 1 : D + 2], 1.0)

            nc.sync.dma_start(
                out=frows_out[g * gp : (g + 1) * gp, :].rearrange(
                    "(b p) r -> p b r", p=P
                ),
                in_=frow_grp[:],
            )

    nc.compile()
    return nc


K2_GRP = 8  # row-chunks per DMA in the caption pass


def _build_k2(rchunks: int):
    """Caption pass: G = W @ rows (fp8), then pooled = (G @ E.T - s)/count.

    The tiny per-caption scalars (lse sum, count) are summed on host from
    k1's returned lse values and fed in as `sx`."""
    _prefer_ln_exp_table()
    nc = bacc.Bacc("TRN2", target_bir_lowering=False, debug=False,
                   num_devices=N_CORES)
    ct_rows = rchunks * P

    ct_in = nc.dram_tensor("ct", [ct_rows, D], FP8, kind="ExternalInput")
    wt_in = nc.dram_tensor("wt", [P, rchunks * C_LOC], FP8, kind="ExternalInput")
    et_in = nc.dram_tensor("et", [P, DK, C_TOT], BF16, kind="ExternalInput")
    sx_in = nc.dram_tensor("sx", [C_LOC, 2], F32, kind="ExternalInput")
    pooled_out = nc.dram_tensor("pooled", [C_LOC, C_TOT], F32, kind="ExternalOutput")
    counts_out = nc.dram_tensor("counts", [C_LOC, 1], F32, kind="ExternalOutput")

    with tile.TileContext(nc) as tc, ExitStack() as ctx:
        outer = ctx.enter_context(tc.tile_pool(name="outer", bufs=1))
        psum_acc = ctx.enter_context(tc.tile_pool(name="psum_acc", bufs=1, space="PSUM"))

        wt_sb = outer.tile([P, rchunks * C_LOC], FP8)
        nc.sync.dma_start(out=wt_sb[:], in_=wt_in[:])
        et_sb = outer.tile([P, DK, C_TOT], BF16)
        nc.sync.dma_start(out=et_sb[:], in_=et_in[:])
        ident = outer.tile([P, P], BF16)
        make_identity(nc, ident[:])

        g_ps = psum_acc.tile([P, D], F32)

        with ExitStack() as lctx:
            gpool = lctx.enter_context(tc.tile_pool(name="rows", bufs=3))
            assert rchunks % K2_GRP == 0
            gp = K2_GRP * P
            for g in range(rchunks // K2_GRP):
                row_grp = gpool.tile([P, K2_GRP, D], FP8, tag="r")
                nc.sync.dma_start(
                    out=row_grp[:],
                    in_=ct_in[g * gp : (g + 1) * gp, :].rearrange(
                        "(b p) r -> p b r", p=P
                    ),
                )
                for b in range(K2_GRP):
                    r = g * K2_GRP + b
                    wr = wt_sb[:, r * C_LOC : (r + 1) * C_LOC]
                    nc.tensor.matmul(
                        out=g_ps[:], lhsT=wr, rhs=row_grp[:, b, :],
                        start=(r == 0), stop=(r == rchunks - 1),
                    )

        with ExitStack() as fctx:
            fpool = fctx.enter_context(tc.tile_pool(name="fin", bufs=1))
            fpsum = fctx.enter_context(tc.tile_pool(name="fps", bufs=1, space="PSUM"))

            g_sb = fpool.tile([P, D], BF16)
            nc.vector.tensor_copy(out=g_sb[:], in_=g_ps[:])
            x_sb = fpool.tile([P, 2], F32)
            nc.sync.dma_start(out=x_sb[:], in_=sx_in[:])

            gt_ps = fpsum.tile([P, D], BF16)
            for k in range(DK):
                nc.tensor.transpose(
                    out=gt_ps[:, k * P : (k + 1) * P],
                    in_=g_sb[:, k * P : (k + 1) * P],
                    identity=ident[:],
                )
            gt_sb = fpool.tile([P, D], BF16)
            nc.vector.tensor_copy(out=gt_sb[:], in_=gt_ps[:])

            p_ps = fpsum.tile([P, C_TOT], F32)
            for k in range(DK):
                for h in range(2):
                    nc.tensor.matmul(
                        out=p_ps[:, h * 512 : (h + 1) * 512],
                        lhsT=gt_sb[:, k * P : (k + 1) * P],
                        rhs=et_sb[:, k, h * 512 : (h + 1) * 512],
                        start=(k == 0),
                        stop=(k == DK - 1),
                    )

            cnt_cl = fpool.tile([P, 1], F32)
            nc.vector.tensor_scalar_max(cnt_cl[:], x_sb[:, 1:2], 1.0)
            rec = fpool.tile([P, 1], F32)
            nc.vector.reciprocal(out=rec[:], in_=cnt_cl[:])
            pooled_sb = fpool.tile([P, C_TOT], F32)
            nc.vector.tensor_scalar(
                out=pooled_sb[:],
                in0=p_ps[:],
                scalar1=x_sb[:, 0:1],
                scalar2=rec[:, 0:1],
                op0=ALU.subtract,
                op1=ALU.mult,
            )
            nc.sync.dma_start(out=pooled_out[:], in_=pooled_sb[:])
            nc.sync.dma_start(out=counts_out[:], in_=x_sb[:, 1:2])

    nc.compile()
    return nc


def _round_up(x, m):
    return ((x + m - 1) // m) * m


def _prep(adapter_feats, caption_embed, logit_scale, v2p_map,
          caption2point_idx, segment_ids):
    af = np.asarray(adapter_feats, dtype=np.float32)
    table_bf16 = af.astype(ml_dtypes.bfloat16)

    e = np.asarray(caption_embed, dtype=np.float32)
    et = np.ascontiguousarray(
        e.T.reshape(DK, P, C_TOT).transpose(1, 0, 2)
    ).astype(ml_dtypes.bfloat16)
    ls = np.array([[float(np.asarray(logit_scale))]], dtype=np.float32)

    v2p = np.asarray(v2p_map).astype(np.int64)
    c2p = np.asarray(caption2point_idx).astype(np.int64)
    seg = np.asarray(segment_ids).astype(np.int64)
    n_pts = v2p.shape[0]

    # ---- k1: pre-gathered per-point rows, point-sharded -------------------
    pt_loc = (n_pts + N_CORES - 1) // N_CORES
    n_pt_pad = _round_up(pt_loc, P * K1_GRP)
    t1 = n_pt_pad // P
    feats = table_bf16[v2p]                      # [n_pts, D] host gather
    k1_maps = []
    for i in range(N_CORES):
        lo = i * pt_loc
        hi = min(lo + pt_loc, n_pts)
        pf = np.ones((n_pt_pad, D), dtype=ml_dtypes.bfloat16)
        pf[: hi - lo] = feats[lo:hi]
        k1_maps.append({"pf": pf, "et": et, "ls": ls})

    # ---- k2: per-caption-range compacted rows + multiplicity matrices -----
    bounds = np.searchsorted(seg, np.arange(0, C_TOT + 1, C_LOC))
    p_row = c2p + (c2p // pt_loc) * (n_pt_pad - pt_loc)  # row in concat frows

    k2_meta = []
    ct_need = []
    rchunks_list = []
    for i in range(N_CORES):
        lo, hi = int(bounds[i]), int(bounds[i + 1])
        uniq, inv = np.unique(p_row[lo:hi], return_inverse=True)
        ct_need.append(uniq)
        rchunks_list.append(_round_up(max(len(uniq), 1), P * K2_GRP) // P)
        k2_meta.append((inv, (seg[lo:hi] - i * C_LOC).astype(np.int64)))
    rchunks = max(rchunks_list)
    ct_rows = rchunks * P

    k2_inputs = []
    for i in range(N_CORES):
        inv, seg_loc = k2_meta[i]
        wt = np.zeros((ct_rows, C_LOC), dtype=np.float32)
        np.add.at(wt, (inv, seg_loc), 1.0)
        # SBUF layout: wt_sb[p, chunk*C_LOC + c] = W^T[chunk*P + p, c]
        wt = np.ascontiguousarray(
            wt.reshape(rchunks, P, C_LOC).transpose(1, 0, 2).reshape(P, rchunks * C_LOC)
        ).astype(NP_FP8)
        k2_inputs.append({"wt": wt, "et": et})

    dims = dict(t1=t1, rchunks=rchunks, ct_rows=ct_rows,
                pt_loc=pt_loc, n_pt_pad=n_pt_pad)
    return dims, k1_maps, k2_inputs, ct_need, (seg, p_row, bounds)


def kernel(adapter_feats, caption_embed, logit_scale, v2p_map,
           caption2point_idx, segment_ids, num_captions):
    assert int(num_captions) == C_TOT
    dims, k1_maps, k2_inputs, ct_need, (seg, p_row, bounds) = _prep(
        adapter_feats, caption_embed, logit_scale, v2p_map,
        caption2point_idx, segment_ids,
    )
    key1 = ("k1", dims["t1"])
    if key1 not in _BUILD_CACHE:
        _BUILD_CACHE[key1] = _build_k1(dims["t1"])
    key2 = ("k2", dims["rchunks"])
    if key2 not in _BUILD_CACHE:
        _BUILD_CACHE[key2] = _build_k2(dims["rchunks"])

    tr1 = {"trace": True, "tmpdir": "/tmp/caption_prof_k1"} if TRACE else {}
    res1 = run_bass_kernel_spmd(
        _BUILD_CACHE[key1], k1_maps, list(range(N_CORES)), **tr1
    )
    frows_full = np.concatenate(
        [res1.results[i]["frows"] for i in range(N_CORES)], axis=0
    )

    # per-caption scalar sums on host from k1's lse column
    lse_t = frows_full[p_row, D].astype(np.float64)
    s_c = np.bincount(seg, weights=lse_t, minlength=C_TOT).astype(np.float32)
    counts = np.bincount(seg, minlength=C_TOT).astype(np.float32)
    fhat8_full = frows_full[:, 0:D].astype(NP_FP8)

    k2_maps = []
    for i in range(N_CORES):
        ct = np.zeros((dims["ct_rows"], D), dtype=NP_FP8)
        ct[: len(ct_need[i])] = fhat8_full[ct_need[i]]
        sx = np.stack(
            [s_c[i * C_LOC : (i + 1) * C_LOC], counts[i * C_LOC : (i + 1) * C_LOC]],
            axis=1,
        ).astype(np.float32)
        k2_maps.append({**k2_inputs[i], "ct": ct, "sx": np.ascontiguousarray(sx)})

    tr2 = {"trace": True, "tmpdir": "/tmp/caption_prof_k2"} if TRACE else {}
    res2 = run_bass_kernel_spmd(
        _BUILD_CACHE[key2], k2_maps, list(range(N_CORES)), **tr2
    )
    if TRACE:
        global LAST_EXEC_NS
        LAST_EXEC_NS = (res1.exec_time_ns, res2.exec_time_ns)
    pooled = np.concatenate(
        [res2.results[i]["pooled"] for i in range(N_CORES)], axis=0
    ).astype(np.float32)
    counts = np.concatenate(
        [res2.results[i]["counts"].reshape(-1) for i in range(N_CORES)]
    ).astype(np.float32)
    return pooled, counts


# revision 24
# speedup vs baseline: 1.1876x; 1.1876x over previous
"""Trainium2 Bass kernel for nn_CaptionHead (segment_reduce).

Math (reference):
    feats = adapter_feats[v2p_map]                      # [N_pts, D]
    fhat  = feats / max(||feats||, eps)
    scores = (fhat @ E.T) * exp(logit_scale)            # [N_pts, C]
    logp = log_softmax(scores, -1)
    pooled[c] = mean_{t in seg c} logp[p_t]             # [C, C]
    counts[c] = |seg c|

Key algebraic restructuring: with s = scale * fhat,
    logp[p, :] = (s_p @ E.T) - lse_p,   lse_p = log(sum_c exp(s_p . e_c))
so the segment mean decomposes into
    pooled[c] = ( (sum_t s_{p_t}) @ E.T - sum_t lse_{p_t} ) / count_c
which needs only per-point rows + one [C_loc, D] accumulator per core —
the [N_pts, C] log-prob matrix is never materialized.  The segment sum
itself is a matmul against a (host-built) caption-x-row multiplicity
matrix W, so the device never does an indexed gather at all.

Two SPMD passes over 8 cores (host does the indexing/sharding between):
  k1 (point-parallel): each core streams its N_pts/8 pre-gathered voxel
     rows, normalizes, computes lse via an on-chip matmul vs the caption
     embeddings, and writes [fhat*scale | lse | 1] rows (514 bf16) back.
     Point-parallel means every point's lse is computed exactly once
     (caption-parallel would recompute shared points, T/N_pts = 1.75x).
  k2 (caption-parallel): each core owns 128 captions and computes
     [G | X] = W @ [rows], pooled = (G @ E.T - lse_sum) / count, where
     W[c, r] = multiplicity of compacted row r in caption c's segment.
"""

import math
from contextlib import ExitStack

import ml_dtypes
import numpy as np

import concourse.bacc as bacc
import concourse.bass as bass
import concourse.tile as tile
from concourse import mybir
from concourse.bass_utils import run_bass_kernel_spmd
from concourse.masks import make_identity

N_CORES = 8
P = 128
D = 512
C_TOT = 1024
C_LOC = C_TOT // N_CORES
DK = D // P            # 128-row d-chunks
ROW = D + 2            # frow width: fhat | lse | 1

BF16 = mybir.dt.bfloat16
F32 = mybir.dt.float32
FP8 = mybir.dt.float8e4
NP_FP8 = mybir.dt.np(FP8)
AF = mybir.ActivationFunctionType
ALU = mybir.AluOpType

_BUILD_CACHE: dict = {}

# Set by a test harness (with an NTFF profile hook installed) to collect
# per-pass HW exec times; leave False in normal use.
TRACE = False
LAST_EXEC_NS = None


def _prefer_ln_exp_table():
    """Make the act-table-set selector favor `natural_log_exp_and_others`,
    which contains both Ln and Exp — otherwise Bacc alternates between the
    Ln-only and Exp-only sets, inserting an ACT table load (~2.7us) around
    nearly every activation."""
    import concourse.bacc as _bacc_mod

    orig = _bacc_mod.get_activation_tables
    if getattr(orig, "_lnexp_patched", False):
        return

    def patched(arch):
        t = dict(orig(arch))
        pref = "natural_log_exp_and_others"
        if pref in t:
            # Keep set order/indices intact (walrus emits the index), but
            # make `pref` the only set advertising Ln/Exp.
            drop = {AF.Ln, AF.Exp}
            t = {k: (v if k == pref else (set(v) - drop)) for k, v in t.items()}
        return t

    patched._lnexp_patched = True
    _bacc_mod.get_activation_tables = patched


K1_GRP = 4  # point-tiles per DMA/activation group


def _build_k1(t1: int):
    """Point pass: stream pre-gathered rows -> normalize -> lse -> frows.

    Tiles are processed in groups of K1_GRP: one input DMA, one output DMA,
    and batched [P, K1_GRP] activations (Ln / Exp) per group, so neither
    the SP DMA-issue cost nor the ACT per-op overhead is paid per tile.
    """
    _prefer_ln_exp_table()
    nc = bacc.Bacc("TRN2", target_bir_lowering=False, debug=False,
                   num_devices=N_CORES)
    assert t1 % K1_GRP == 0
    n_pt = t1 * P
    n_grp = t1 // K1_GRP
    gp = K1_GRP * P

    pf_in = nc.dram_tensor("pf", [n_pt, D], BF16, kind="ExternalInput")
    # d-major copy: row g*P+dd holds, at col (j*DK+k)*P+tt, f[g*gp + j*P + tt, k*P + dd]
    pft_in = nc.dram_tensor("pft", [n_grp * P, K1_GRP * DK * P], BF16,
                            kind="ExternalInput")
    et_in = nc.dram_tensor("et", [P, DK, C_TOT], BF16, kind="ExternalInput")
    ls_in = nc.dram_tensor("ls", [1, 1], F32, kind="ExternalInput")
    frows_out = nc.dram_tensor("frows", [n_pt, ROW], BF16, kind="ExternalOutput")

    with tile.TileContext(nc) as tc, ExitStack() as ctx:
        outer = ctx.enter_context(tc.tile_pool(name="outer", bufs=1))
        et_sb = outer.tile([P, DK, C_TOT], BF16)
        nc.sync.dma_start(out=et_sb[:], in_=et_in[:])
        ls_sb = outer.tile([P, 1], F32)
        nc.gpsimd.dma_start(out=ls_sb[:], in_=ls_in[:].to_broadcast([P, 1]))

        gpool = ctx.enter_context(tc.tile_pool(name="gath", bufs=3))
        tpool = ctx.enter_context(tc.tile_pool(name="gatht", bufs=3))
        spool = ctx.enter_context(tc.tile_pool(name="work", bufs=3))
        stat = ctx.enter_context(tc.tile_pool(name="stat", bufs=3))
        opool = ctx.enter_context(tc.tile_pool(name="orow", bufs=3))
        psum_s = ctx.enter_context(tc.tile_pool(name="ps_s", bufs=3, space="PSUM"))

        for g in range(n_grp):
            f_grp = gpool.tile([P, K1_GRP, D], BF16, tag="f")
            nc.sync.dma_start(
                out=f_grp[:],
                in_=pf_in[g * gp : (g + 1) * gp, :].rearrange(
                    "(b p) d -> p b d", p=P
                ),
            )
            ft_grp = tpool.tile([P, K1_GRP * DK * P], BF16, tag="ft")
            nc.sync.dma_start(
                out=ft_grp[:], in_=pft_in[g * P : (g + 1) * P, :]
            )
            frow_grp = opool.tile([P, K1_GRP, ROW], BF16, tag="frow")

            # batched sum-of-squares -> rns = exp(ls - 0.5*ln(ss))
            ss4 = stat.tile([P, K1_GRP], F32, tag="ss")
            for jj in range(K1_GRP):
                fsq = spool.tile([P, D], BF16, tag="fsq")
                nc.gpsimd.tensor_mul(fsq[:], f_grp[:, jj, :], f_grp[:, jj, :])
                nc.vector.reduce_sum(
                    out=ss4[:, jj : jj + 1], in_=fsq[:], axis=mybir.AxisListType.X
                )
            lnss4 = stat.tile([P, K1_GRP], F32, tag="lnss")
            nc.scalar.activation(out=lnss4[:], in_=ss4[:], func=AF.Ln)
            rns4 = stat.tile([P, K1_GRP], F32, tag="rns")
            nc.scalar.activation(
                out=rns4[:], in_=lnss4[:], func=AF.Exp, scale=-0.5, bias=ls_sb[:, 0:1]
            )

            se4 = stat.tile([P, K1_GRP], F32, tag="se")
            for jj in range(K1_GRP):
                frow = frow_grp[:, jj, :]
                nc.vector.tensor_scalar_mul(
                    frow[:, 0:D], f_grp[:, jj, :], rns4[:, jj : jj + 1]
                )

                # raw scores from the host-transposed layout (no PE transpose)
                s_ps = psum_s.tile([P, C_TOT], F32, tag="sps")
                for k in range(DK):
                    for h in range(2):
                        nc.tensor.matmul(
                            out=s_ps[:, h * 512 : (h + 1) * 512],
                            lhsT=ft_grp[:, (jj * DK + k) * P : (jj * DK + k + 1) * P],
                            rhs=et_sb[:, k, h * 512 : (h + 1) * 512],
                            start=(k == 0),
                            stop=(k == DK - 1),
                        )
                # normalize the raw scores in place, then exp+accumulate
                nc.vector.tensor_scalar_mul(s_ps[:], s_ps[:], rns4[:, jj : jj + 1])
                nc.scalar.activation(
                    out=s_ps[:], in_=s_ps[:], func=AF.Exp,
                    accum_out=se4[:, jj : jj + 1],
                )

            # batched lse + ones columns (strided APs into the group tile)
            nc.scalar.activation(
                out=frow_grp[:, :, D : D + 1].rearrange("p b one -> p (b one)"),
                in_=se4[:],
                func=AF.Ln,
            )
            nc.vector.memset(frow_grp[:, :, D + 1 : D + 2], 1.0)

            nc.sync.dma_start(
                out=frows_out[g * gp : (g + 1) * gp, :].rearrange(
                    "(b p) r -> p b r", p=P
                ),
                in_=frow_grp[:],
            )

    nc.compile()
    return nc


K2_GRP = 8  # row-chunks per DMA in the caption pass


def _build_k2(rchunks: int):
    """Caption pass: G = W @ rows (fp8), then pooled = (G @ E.T - s)/count.

    The tiny per-caption scalars (lse sum, count) are summed on host from
    k1's returned lse values and fed in as `sx`."""
    _prefer_ln_exp_table()
    nc = bacc.Bacc("TRN2", target_bir_lowering=False, debug=False,
                   num_devices=N_CORES)
    ct_rows = rchunks * P

    ct_in = nc.dram_tensor("ct", [ct_rows, D], FP8, kind="ExternalInput")
    wt_in = nc.dram_tensor("wt", [P, rchunks * C_LOC], FP8, kind="ExternalInput")
    et_in = nc.dram_tensor("et", [P, DK, C_TOT], BF16, kind="ExternalInput")
    sx_in = nc.dram_tensor("sx", [C_LOC, 2], F32, kind="ExternalInput")
    pooled_out = nc.dram_tensor("pooled", [C_LOC, C_TOT], F32, kind="ExternalOutput")
    counts_out = nc.dram_tensor("counts", [C_LOC, 1], F32, kind="ExternalOutput")

    with tile.TileContext(nc) as tc, ExitStack() as ctx:
        outer = ctx.enter_context(tc.tile_pool(name="outer", bufs=1))
        psum_acc = ctx.enter_context(tc.tile_pool(name="psum_acc", bufs=1, space="PSUM"))

        wt_sb = outer.tile([P, rchunks * C_LOC], FP8)
        nc.sync.dma_start(out=wt_sb[:], in_=wt_in[:])
        et_sb = outer.tile([P, DK, C_TOT], BF16)
        nc.sync.dma_start(out=et_sb[:], in_=et_in[:])
        ident = outer.tile([P, P], BF16)
        make_identity(nc, ident[:])

        g_ps = psum_acc.tile([P, D], F32)

        with ExitStack() as lctx:
            gpool = lctx.enter_context(tc.tile_pool(name="rows", bufs=3))
            assert rchunks % K2_GRP == 0
            gp = K2_GRP * P
            for g in range(rchunks // K2_GRP):
                row_grp = gpool.tile([P, K2_GRP, D], FP8, tag="r")
                nc.sync.dma_start(
                    out=row_grp[:],
                    in_=ct_in[g * gp : (g + 1) * gp, :].rearrange(
                        "(b p) r -> p b r", p=P
                    ),
                )
                for b in range(K2_GRP):
                    r = g * K2_GRP + b
                    wr = wt_sb[:, r * C_LOC : (r + 1) * C_LOC]
                    nc.tensor.matmul(
                        out=g_ps[:], lhsT=wr, rhs=row_grp[:, b, :],
                        start=(r == 0), stop=(r == rchunks - 1),
                    )

        with ExitStack() as fctx:
            fpool = fctx.enter_context(tc.tile_pool(name="fin", bufs=1))
            fpsum = fctx.enter_context(tc.tile_pool(name="fps", bufs=1, space="PSUM"))

            g_sb = fpool.tile([P, D], BF16)
            nc.vector.tensor_copy(out=g_sb[:], in_=g_ps[:])
            x_sb = fpool.tile([P, 2], F32)
            nc.sync.dma_start(out=x_sb[:], in_=sx_in[:])

            gt_ps = fpsum.tile([P, D], BF16)
            for k in range(DK):
                nc.tensor.transpose(
                    out=gt_ps[:, k * P : (k + 1) * P],
                    in_=g_sb[:, k * P : (k + 1) * P],
                    identity=ident[:],
                )
            gt_sb = fpool.tile([P, D], BF16)
            nc.vector.tensor_copy(out=gt_sb[:], in_=gt_ps[:])

            p_ps = fpsum.tile([P, C_TOT], F32)
            for k in range(DK):
                for h in range(2):
                    nc.tensor.matmul(
                        out=p_ps[:, h * 512 : (h + 1) * 512],
                        lhsT=gt_sb[:, k * P : (k + 1) * P],
                        rhs=et_sb[:, k, h * 512 : (h + 1) * 512],
                        start=(k == 0),
                        stop=(k == DK - 1),
                    )

            cnt_cl = fpool.tile([P, 1], F32)
            nc.vector.tensor_scalar_max(cnt_cl[:], x_sb[:, 1:2], 1.0)
            rec = fpool.tile([P, 1], F32)
            nc.vector.reciprocal(out=rec[:], in_=cnt_cl[:])
            pooled_sb = fpool.tile([P, C_TOT], F32)
            nc.vector.tensor_scalar(
                out=pooled_sb[:],
                in0=p_ps[:],
                scalar1=x_sb[:, 0:1],
                scalar2=rec[:, 0:1],
                op0=ALU.subtract,
                op1=ALU.mult,
            )
            nc.sync.dma_start(out=pooled_out[:], in_=pooled_sb[:])
            nc.sync.dma_start(out=counts_out[:], in_=x_sb[:, 1:2])

    nc.compile()
    return nc


def _round_up(x, m):
    return ((x + m - 1) // m) * m


def _prep(adapter_feats, caption_embed, logit_scale, v2p_map,
          caption2point_idx, segment_ids):
    af = np.asarray(adapter_feats, dtype=np.float32)
    table_bf16 = af.astype(ml_dtypes.bfloat16)

    e = np.asarray(caption_embed, dtype=np.float32)
    et = np.ascontiguousarray(
        e.T.reshape(DK, P, C_TOT).transpose(1, 0, 2)
    ).astype(ml_dtypes.bfloat16)
    ls = np.array([[float(np.asarray(logit_scale))]], dtype=np.float32)

    v2p = np.asarray(v2p_map).astype(np.int64)
    c2p = np.asarray(caption2point_idx).astype(np.int64)
    seg = np.asarray(segment_ids).astype(np.int64)
    n_pts = v2p.shape[0]

    # ---- k1: pre-gathered per-point rows, point-sharded -------------------
    pt_loc = (n_pts + N_CORES - 1) // N_CORES
    n_pt_pad = _round_up(pt_loc, P * K1_GRP)
    t1 = n_pt_pad // P
    feats = table_bf16[v2p]                      # [n_pts, D] host gather
    n_grp = t1 // K1_GRP
    k1_maps = []
    for i in range(N_CORES):
        lo = i * pt_loc
        hi = min(lo + pt_loc, n_pts)
        pf = np.ones((n_pt_pad, D), dtype=ml_dtypes.bfloat16)
        pf[: hi - lo] = feats[lo:hi]
        # d-major layout (verified): pft[g*P+dd, (j*DK+k)*P+tt] = pf[g*gp+j*P+tt, k*P+dd]
        pft = np.ascontiguousarray(
            pf.reshape(n_grp, K1_GRP, P, DK, P)   # [g, j, tt, k, dd]
            .transpose(0, 4, 1, 3, 2)             # [g, dd, j, k, tt]
            .reshape(n_grp * P, K1_GRP * DK * P)
        )
        k1_maps.append({"pf": pf, "pft": pft, "et": et, "ls": ls})

    # ---- k2: per-caption-range compacted rows + multiplicity matrices -----
    bounds = np.searchsorted(seg, np.arange(0, C_TOT + 1, C_LOC))
    p_row = c2p + (c2p // pt_loc) * (n_pt_pad - pt_loc)  # row in concat frows

    k2_meta = []
    ct_need = []
    rchunks_list = []
    for i in range(N_CORES):
        lo, hi = int(bounds[i]), int(bounds[i + 1])
        uniq, inv = np.unique(p_row[lo:hi], return_inverse=True)
        ct_need.append(uniq)
        rchunks_list.append(_round_up(max(len(uniq), 1), P * K2_GRP) // P)
        k2_meta.append((inv, (seg[lo:hi] - i * C_LOC).astype(np.int64)))
    rchunks = max(rchunks_list)
    ct_rows = rchunks * P

    k2_inputs = []
    for i in range(N_CORES):
        inv, seg_loc = k2_meta[i]
        wt = np.zeros((ct_rows, C_LOC), dtype=np.float32)
        np.add.at(wt, (inv, seg_loc), 1.0)
        # SBUF layout: wt_sb[p, chunk*C_LOC + c] = W^T[chunk*P + p, c]
        wt = np.ascontiguousarray(
            wt.reshape(rchunks, P, C_LOC).transpose(1, 0, 2).reshape(P, rchunks * C_LOC)
        ).astype(NP_FP8)
        k2_inputs.append({"wt": wt, "et": et})

    dims = dict(t1=t1, rchunks=rchunks, ct_rows=ct_rows,
                pt_loc=pt_loc, n_pt_pad=n_pt_pad)
    return dims, k1_maps, k2_inputs, ct_need, (seg, p_row, bounds)


def kernel(adapter_feats, caption_embed, logit_scale, v2p_map,
           caption2point_idx, segment_ids, num_captions):
    assert int(num_captions) == C_TOT
    dims, k1_maps, k2_inputs, ct_need, (seg, p_row, bounds) = _prep(
        adapter_feats, caption_embed, logit_scale, v2p_map,
        caption2point_idx, segment_ids,
    )
    key1 = ("k1", dims["t1"])
    if key1 not in _BUILD_CACHE:
        _BUILD_CACHE[key1] = _build_k1(dims["t1"])
    key2 = ("k2", dims["rchunks"])
    if key2 not in _BUILD_CACHE:
        _BUILD_CACHE[key2] = _build_k2(dims["rchunks"])

    tr1 = {"trace": True, "tmpdir": "/tmp/caption_prof_k1"} if TRACE else {}
    res1 = run_bass_kernel_spmd(
        _BUILD_CACHE[key1], k1_maps, list(range(N_CORES)), **tr1
    )
    frows_full = np.concatenate(
        [res1.results[i]["frows"] for i in range(N_CORES)], axis=0
    )

    # per-caption scalar sums on host from k1's lse column
    lse_t = frows_full[p_row, D].astype(np.float64)
    s_c = np.bincount(seg, weights=lse_t, minlength=C_TOT).astype(np.float32)
    counts = np.bincount(seg, minlength=C_TOT).astype(np.float32)
    fhat8_full = frows_full[:, 0:D].astype(NP_FP8)

    k2_maps = []
    for i in range(N_CORES):
        ct = np.zeros((dims["ct_rows"], D), dtype=NP_FP8)
        ct[: len(ct_need[i])] = fhat8_full[ct_need[i]]
        sx = np.stack(
            [s_c[i * C_LOC : (i + 1) * C_LOC], counts[i * C_LOC : (i + 1) * C_LOC]],
            axis=1,
        ).astype(np.float32)
        k2_maps.append({**k2_inputs[i], "ct": ct, "sx": np.ascontiguousarray(sx)})

    tr2 = {"trace": True, "tmpdir": "/tmp/caption_prof_k2"} if TRACE else {}
    res2 = run_bass_kernel_spmd(
        _BUILD_CACHE[key2], k2_maps, list(range(N_CORES)), **tr2
    )
    if TRACE:
        global LAST_EXEC_NS
        LAST_EXEC_NS = (res1.exec_time_ns, res2.exec_time_ns)
    pooled = np.concatenate(
        [res2.results[i]["pooled"] for i in range(N_CORES)], axis=0
    ).astype(np.float32)
    counts = np.concatenate(
        [res2.results[i]["counts"].reshape(-1) for i in range(N_CORES)]
    ).astype(np.float32)
    return pooled, counts
